# revision 17
# baseline (speedup 1.0000x reference)
"""DecisionGate (moe_routing) Bass kernel for 8 TRN2 NeuronCores.

Problem (hardcoded):
    x         [4096, 64]  f32
    act       [4096, 512] f32
    batch_inds[4096]      int64 (unused by the dense formulation)
Returns (g, mask, dispatched):
    g          [4096, 64]        f32   = 1 / (1 + x^4)
    mask       [4096, 64]        bool  = g >= 0.5
    dispatched [4096, 64, 512]   f32   = where(mask, g, 0)[:, :, None] * act[:, None, :]

Sharding: data parallel over batch B across 8 cores (512 rows/core).
Each core is fully independent (row-wise compute, no collectives).
The dominant cost is streaming the 64MB/core `dispatched` output to HBM
(8 cores x 64MB = 512MB), so the kernel is structured as: act shard
resident in SBUF, outer-product tiles produced by VectorE (tensor_scalar
with per-partition scalar, fp32 2x mode) and ScalarE (activation Copy
with per-partition scale) in parallel, stored with 4MB HWDGE DMAs on the
SP ring, triple buffered; input loads and the small g store ride the
SWDGE (gpsimd) ring so they never queue behind the output stream.

Measured on HW (R-loop slope timing): ~206us/core steady state
= 93% of the 358GB/s per-core HBM roofline; a stores-only variant of
the same structure measures the same ~206us, i.e. compute (78us) is
fully hidden and the kernel is pinned at its DMA-write bound.
"""

import numpy as np

import concourse.bacc as bacc
import concourse.mybir as mybir
from concourse.tile import TileContext
from concourse.bass_utils import run_bass_kernel_spmd

N_CORES = 8
B, P, D = 4096, 64, 512
BS = B // N_CORES          # 512 batch rows per core
NP = 128                   # SBUF partitions
NG = BS // NP              # 4 groups of 128 rows per core
PB = 16                    # p-block per output tile -> [128, PB*D] = 4MB tiles
THRES = 0.5

_cache = {}


def _build(reps=1, pb=PB, dbufs=3, mode="full", dma_engines=("sync",),
           io_engine="gpsimd", fine=False, mul_split=5):
    f32 = mybir.dt.float32
    nc = bacc.Bacc(
        "TRN2",
        target_bir_lowering=False,
        debug=False,
        enable_asserts=False,
        num_devices=N_CORES,
    )
    x_d = nc.declare_dram_parameter("x", [BS, P], f32, isOutput=False)
    a_d = nc.declare_dram_parameter("act", [BS, D], f32, isOutput=False)
    g_d = nc.declare_dram_parameter("g", [BS, P], f32, isOutput=True)
    disp_d = nc.declare_dram_parameter("disp", [BS, P, D], f32, isOutput=True)

    # DRAM views: row (n*128 + p) -> partition p, free group n
    x_v = x_d[:].rearrange("(n p) m -> p n m", p=NP)        # [128, NG, 64]
    a_v = a_d[:].rearrange("(n p) d -> p n d", p=NP)        # [128, NG, 512]
    g_v = g_d[:].rearrange("(n p) m -> p n m", p=NP)

    with TileContext(nc) as tc:
        with (
            tc.tile_pool(name="small", bufs=1 if reps == 1 else 2) as small,
            tc.tile_pool(name="disp", bufs=dbufs) as dpool,
        ):
            def body(_i=None):
                x_t = small.tile([NP, NG * P], f32, tag="x")
                a_t = small.tile([NP, NG * D], f32, tag="a")
                t_t = small.tile([NP, NG * P], f32, tag="t")
                g_t = small.tile([NP, NG * P], f32, tag="g")
                w_t = small.tile([NP, NG * P], f32, tag="w")

                io_eng = getattr(nc, io_engine)
                io_eng.dma_start(
                    out=x_t[:].rearrange("p (n m) -> p n m", n=NG), in_=x_v
                )
                if fine:
                    for n in range(NG):
                        io_eng.dma_start(
                            out=a_t[:, n * D:(n + 1) * D], in_=a_v[:, n, :]
                        )
                    gslices = [
                        (slice(n * P, (n + 1) * P),) for n in range(NG)
                    ]
                else:
                    io_eng.dma_start(
                        out=a_t[:].rearrange("p (n d) -> p n d", n=NG), in_=a_v
                    )
                    gslices = [(slice(0, NG * P),)]

                # g = 1 / (1 + x^4); w = (g >= thres) * g
                for (sl,) in gslices:
                    nc.scalar.activation(
                        out=t_t[:, sl], in_=x_t[:, sl],
                        func=mybir.ActivationFunctionType.Square,
                    )
                    nc.scalar.activation(
                        out=t_t[:, sl], in_=t_t[:, sl],
                        func=mybir.ActivationFunctionType.Square,
                    )
                    nc.vector.tensor_scalar_add(
                        out=t_t[:, sl], in0=t_t[:, sl], scalar1=1.0
                    )
                    nc.vector.reciprocal(out=g_t[:, sl], in_=t_t[:, sl])
                    nc.vector.scalar_tensor_tensor(
                        out=w_t[:, sl],
                        in0=g_t[:, sl],
                        scalar=THRES,
                        in1=g_t[:, sl],
                        op0=mybir.AluOpType.is_ge,
                        op1=mybir.AluOpType.mult,
                    )
                io_eng.dma_start(
                    out=g_v, in_=g_t[:].rearrange("p (n m) -> p n m", n=NG)
                )

                # dispatched[n*128+q, p, :] = w[q, n*64+p] * act_row
                dma_i = 0
                for n in range(NG):
                    a_n = a_t[:, n * D:(n + 1) * D]
                    for j in range(P // pb):
                        d_t = dpool.tile([NP, pb * D], f32, tag="d")
                        if mode != "dma_only":
                            for k in range(pb):
                                p = j * pb + k
                                w_col = w_t[:, n * P + p:n * P + p + 1]
                                out_sl = d_t[:, k * D:(k + 1) * D]
                                # DVE fp32 tensor_scalar runs 2x/cycle
                                # @0.96GHz, ACT 1x @1.2GHz -> split ~10:6
                                if k % 8 < mul_split:
                                    nc.vector.tensor_scalar_mul(
                                        out=out_sl, in0=a_n, scalar1=w_col
                                    )
                                else:
                                    nc.scalar.mul(
                                        out=out_sl, in_=a_n, mul=w_col
                                    )
                        else:
                            # touch the tile once so it has a producer
                            nc.vector.tensor_scalar_mul(
                                out=d_t[:, 0:D], in0=a_n,
                                scalar1=w_t[:, 0:1],
                            )
                        if mode == "full_split2":
                            # both rings stream halves of the same tile
                            h = pb // 2
                            nc.sync.dma_start(
                                out=disp_d[
                                    n * NP:(n + 1) * NP, j * pb:j * pb + h, :
                                ],
                                in_=d_t[:, :h * D].rearrange(
                                    "q (a b) -> q a b", a=h
                                ),
                            )
                            nc.scalar.dma_start(
                                out=disp_d[
                                    n * NP:(n + 1) * NP,
                                    j * pb + h:(j + 1) * pb, :
                                ],
                                in_=d_t[:, h * D:].rearrange(
                                    "q (a b) -> q a b", a=h
                                ),
                            )
                        elif mode != "compute_only":
                            eng = getattr(nc, dma_engines[dma_i % len(dma_engines)])
                            dma_i += 1
                            eng.dma_start(
                                out=disp_d[
                                    n * NP:(n + 1) * NP, j * pb:(j + 1) * pb, :
                                ],
                                in_=d_t[:].rearrange("q (a b) -> q a b", a=pb),
                            )
                        else:
                            # tiny consumer so DCE keeps the compute
                            nc.sync.dma_start(
                                out=disp_d[n * NP:(n + 1) * NP, j * pb, :1],
                                in_=d_t[:, :1],
                            )

            if reps == 1:
                body()
            else:
                with tc.For_i(0, reps, 1) as i:
                    body(i)
    nc.compile()
    return nc


def _make_fast_runner(nc):
    """Cached PJRT runner: jit once, keep the inert zero output-operand
    buffers device-resident (every output element is written by the NEFF,
    so no donation/zero-init is needed), avoiding per-call re-tracing and
    the 0.5GB zero re-upload that run_bass_kernel_spmd pays."""
    import jax
    from jax.sharding import Mesh, NamedSharding, PartitionSpec
    from concourse import bass2jax

    bass2jax.install_neuronx_cc_hook()
    partition_name = nc.partition_id_tensor.name if nc.partition_id_tensor else None
    in_names, out_names, out_avals = [], [], []
    for alloc in nc.m.functions[0].allocations:
        if not isinstance(alloc, mybir.MemoryLocationSet):
            continue
        name = alloc.memorylocations[0].name
        if alloc.kind == "ExternalInput":
            if name != partition_name:
                in_names.append(name)
        elif alloc.kind == "ExternalOutput":
            out_avals.append(
                jax.core.ShapedArray(
                    tuple(alloc.tensor_shape), mybir.dt.np(alloc.dtype)
                )
            )
            out_names.append(name)
    all_in_names = list(in_names) + list(out_names)
    if partition_name is not None:
        all_in_names.append(partition_name)

    def _body(*args):
        operands = list(args)
        if partition_name is not None:
            operands.append(bass2jax.partition_id_tensor())
        return tuple(
            bass2jax._bass_exec_p.bind(
                *operands,
                out_avals=tuple(out_avals),
                in_names=tuple(all_in_names),
                out_names=tuple(out_names),
                lowering_input_output_aliases=(),
                sim_require_finite=True,
                sim_require_nnan=True,
                nc=nc,
            )
        )

    devices = jax.devices()[:N_CORES]
    mesh = Mesh(np.asarray(devices), ("core",))
    spec = PartitionSpec("core")
    fn = jax.jit(
        jax.shard_map(
            _body, mesh=mesh,
            in_specs=(spec,) * (len(in_names) + len(out_names)),
            out_specs=(spec,) * len(out_names),
            check_vma=False,
        ),
        keep_unused=True,
    )
    sharding = NamedSharding(mesh, spec)
    zeros = [
        jax.device_put(
            np.zeros((N_CORES * a.shape[0], *a.shape[1:]), a.dtype), sharding
        )
        for a in out_avals
    ]

    def run(x, act):
        args = [jax.device_put(v, sharding) for v in (x, act)]
        outs = fn(*args, *zeros)
        return {n: np.asarray(o) for n, o in zip(out_names, outs)}

    return run


def kernel(x, act, batch_inds=None, _trace=False, _results_out=None, **_kw):
    import os

    x = np.ascontiguousarray(np.asarray(x, dtype=np.float32))
    act = np.ascontiguousarray(np.asarray(act, dtype=np.float32))
    assert x.shape == (B, P) and act.shape == (B, D), (x.shape, act.shape)

    if "nc" not in _cache:
        _cache["nc"] = _build()
    nc = _cache["nc"]

    g = dispatched = None
    # Fast path: cached jit runner. Skipped when a profiling harness asks
    # for traces (BASS_TRACE) so run_bass_kernel_spmd's NTFF hook can fire.
    if not _trace and not os.environ.get("BASS_TRACE"):
        try:
            if "runner" not in _cache:
                _cache["runner"] = _make_fast_runner(nc)
            out = _cache["runner"](x, act)
            g, dispatched = out["g"], out["disp"]
        except Exception:
            _cache.pop("runner", None)

    if g is None:
        in_maps = [
            {
                "x": np.ascontiguousarray(x[i * BS:(i + 1) * BS]),
                "act": np.ascontiguousarray(act[i * BS:(i + 1) * BS]),
            }
            for i in range(N_CORES)
        ]
        try:
            res = run_bass_kernel_spmd(
                nc, in_maps, list(range(N_CORES)), trace=_trace
            )
        except ModuleNotFoundError:
            # axon client without the NTFF profile hook — run untraced
            res = run_bass_kernel_spmd(
                nc, in_maps, list(range(N_CORES)), trace=False
            )
        if _results_out is not None:
            _results_out["bass_results"] = res
        g = np.concatenate([r["g"] for r in res.results], axis=0)
        dispatched = np.concatenate([r["disp"] for r in res.results], axis=0)

    mask = g >= np.float32(THRES)
    return g, mask, dispatched


# revision 25
# speedup vs baseline: 1.0699x; 1.0699x over previous
"""DecisionGate (moe_routing) Bass kernel for 8 TRN2 NeuronCores.

Problem (hardcoded):
    x         [4096, 64]  f32
    act       [4096, 512] f32
    batch_inds[4096]      int64 (unused by the dense formulation)
Returns (g, mask, dispatched):
    g          [4096, 64]        f32   = 1 / (1 + x^4)
    mask       [4096, 64]        bool  = g >= 0.5
    dispatched [4096, 64, 512]   f32   = where(mask, g, 0)[:, :, None] * act[:, None, :]

Sharding: data parallel over batch B across 8 cores (512 rows/core).
Each core is fully independent (row-wise compute, no collectives).
The dominant cost is streaming the 64MB/core `dispatched` output to HBM
(8 cores x 64MB = 512MB), so the kernel is structured as: act shard
resident in SBUF, outer-product tiles produced by VectorE (tensor_scalar
with per-partition scalar, fp32 2x mode) and ScalarE (activation Copy
with per-partition scale) in parallel, stored with 4MB HWDGE DMAs on the
SP ring, triple buffered; input loads and the small g store ride the
SWDGE (gpsimd) ring so they never queue behind the output stream.

Measured on HW (R-loop slope timing): ~206us/core steady state
= 93% of the 358GB/s per-core HBM roofline; a stores-only variant of
the same structure measures the same ~206us, i.e. compute (78us) is
fully hidden and the kernel is pinned at its DMA-write bound.
"""

import numpy as np

import concourse.bacc as bacc
import concourse.mybir as mybir
from concourse.tile import TileContext
from concourse.bass_utils import run_bass_kernel_spmd

N_CORES = 8
B, P, D = 4096, 64, 512
BS = B // N_CORES          # 512 batch rows per core
NP = 128                   # SBUF partitions
NG = BS // NP              # 4 groups of 128 rows per core
PB = 16                    # p-block per output tile -> [128, PB*D] = 4MB tiles
THRES = 0.5

_cache = {}


def _build(reps=1, pb=PB, dbufs=3, mode="full", dma_engines=("sync",),
           io_engine="gpsimd", loads_engine="sync", fine=True, first_split=True,
           mul_split=5):
    f32 = mybir.dt.float32
    nc = bacc.Bacc(
        "TRN2",
        target_bir_lowering=False,
        debug=False,
        enable_asserts=False,
        num_devices=N_CORES,
    )
    x_d = nc.declare_dram_parameter("x", [BS, P], f32, isOutput=False)
    a_d = nc.declare_dram_parameter("act", [BS, D], f32, isOutput=False)
    g_d = nc.declare_dram_parameter("g", [BS, P], f32, isOutput=True)
    disp_d = nc.declare_dram_parameter("disp", [BS, P, D], f32, isOutput=True)

    # DRAM views: row (n*128 + p) -> partition p, free group n
    x_v = x_d[:].rearrange("(n p) m -> p n m", p=NP)        # [128, NG, 64]
    a_v = a_d[:].rearrange("(n p) d -> p n d", p=NP)        # [128, NG, 512]
    g_v = g_d[:].rearrange("(n p) m -> p n m", p=NP)

    with TileContext(nc) as tc:
        with (
            tc.tile_pool(name="small", bufs=1 if reps == 1 else 2) as small,
            tc.tile_pool(name="disp", bufs=dbufs) as dpool,
        ):
            def body(_i=None):
                x_t = small.tile([NP, NG * P], f32, tag="x")
                a_t = small.tile([NP, NG * D], f32, tag="a")
                t_t = small.tile([NP, NG * P], f32, tag="t")
                g_t = small.tile([NP, NG * P], f32, tag="g")
                w_t = small.tile([NP, NG * P], f32, tag="w")

                io_eng = getattr(nc, io_engine)
                ld_eng = getattr(nc, loads_engine) if loads_engine else io_eng
                ld_eng.dma_start(
                    out=x_t[:].rearrange("p (n m) -> p n m", n=NG), in_=x_v
                )
                if fine:
                    for n in range(NG):
                        ld_eng.dma_start(
                            out=a_t[:, n * D:(n + 1) * D], in_=a_v[:, n, :]
                        )
                    gslices = [
                        (slice(n * P, (n + 1) * P),) for n in range(NG)
                    ]
                else:
                    ld_eng.dma_start(
                        out=a_t[:].rearrange("p (n d) -> p n d", n=NG), in_=a_v
                    )
                    gslices = [(slice(0, NG * P),)]

                # g = 1 / (1 + x^4); w = (g >= thres) * g
                for (sl,) in gslices:
                    nc.scalar.activation(
                        out=t_t[:, sl], in_=x_t[:, sl],
                        func=mybir.ActivationFunctionType.Square,
                    )
                    nc.scalar.activation(
                        out=t_t[:, sl], in_=t_t[:, sl],
                        func=mybir.ActivationFunctionType.Square,
                    )
                    nc.vector.tensor_scalar_add(
                        out=t_t[:, sl], in0=t_t[:, sl], scalar1=1.0
                    )
                    nc.vector.reciprocal(out=g_t[:, sl], in_=t_t[:, sl])
                    nc.vector.scalar_tensor_tensor(
                        out=w_t[:, sl],
                        in0=g_t[:, sl],
                        scalar=THRES,
                        in1=g_t[:, sl],
                        op0=mybir.AluOpType.is_ge,
                        op1=mybir.AluOpType.mult,
                    )
                io_eng.dma_start(
                    out=g_v, in_=g_t[:].rearrange("p (n m) -> p n m", n=NG)
                )

                # dispatched[n*128+q, p, :] = w[q, n*64+p] * act_row
                dma_i = 0
                for n in range(NG):
                    a_n = a_t[:, n * D:(n + 1) * D]
                    for j in range(P // pb):
                        if first_split and n == 0 and j == 0:
                            # 1MB first stores: the stream starts after just
                            # 4 multiplies instead of 16 (shorter head)
                            for jj in range(4):
                                d_s = dpool.tile([NP, 4 * D], f32, tag="d0")
                                for k in range(4):
                                    p = jj * 4 + k
                                    w_col = w_t[:, p:p + 1]
                                    out_sl = d_s[:, k * D:(k + 1) * D]
                                    if k % 2 == 0:
                                        nc.vector.tensor_scalar_mul(
                                            out=out_sl, in0=a_n, scalar1=w_col
                                        )
                                    else:
                                        nc.scalar.mul(
                                            out=out_sl, in_=a_n, mul=w_col
                                        )
                                nc.sync.dma_start(
                                    out=disp_d[0:NP, jj * 4:(jj + 1) * 4, :],
                                    in_=d_s[:].rearrange(
                                        "q (a b) -> q a b", a=4
                                    ),
                                )
                            continue
                        d_t = dpool.tile([NP, pb * D], f32, tag="d")
                        if mode != "dma_only":
                            for k in range(pb):
                                p = j * pb + k
                                w_col = w_t[:, n * P + p:n * P + p + 1]
                                out_sl = d_t[:, k * D:(k + 1) * D]
                                # DVE fp32 tensor_scalar runs 2x/cycle
                                # @0.96GHz, ACT 1x @1.2GHz -> split ~10:6
                                if k % 8 < mul_split:
                                    nc.vector.tensor_scalar_mul(
                                        out=out_sl, in0=a_n, scalar1=w_col
                                    )
                                else:
                                    nc.scalar.mul(
                                        out=out_sl, in_=a_n, mul=w_col
                                    )
                        else:
                            # touch the tile once so it has a producer
                            nc.vector.tensor_scalar_mul(
                                out=d_t[:, 0:D], in0=a_n,
                                scalar1=w_t[:, 0:1],
                            )
                        if mode == "full_split2":
                            # both rings stream halves of the same tile
                            h = pb // 2
                            nc.sync.dma_start(
                                out=disp_d[
                                    n * NP:(n + 1) * NP, j * pb:j * pb + h, :
                                ],
                                in_=d_t[:, :h * D].rearrange(
                                    "q (a b) -> q a b", a=h
                                ),
                            )
                            nc.scalar.dma_start(
                                out=disp_d[
                                    n * NP:(n + 1) * NP,
                                    j * pb + h:(j + 1) * pb, :
                                ],
                                in_=d_t[:, h * D:].rearrange(
                                    "q (a b) -> q a b", a=h
                                ),
                            )
                        elif mode != "compute_only":
                            eng = getattr(nc, dma_engines[dma_i % len(dma_engines)])
                            dma_i += 1
                            eng.dma_start(
                                out=disp_d[
                                    n * NP:(n + 1) * NP, j * pb:(j + 1) * pb, :
                                ],
                                in_=d_t[:].rearrange("q (a b) -> q a b", a=pb),
                            )
                        else:
                            # tiny consumer so DCE keeps the compute
                            nc.sync.dma_start(
                                out=disp_d[n * NP:(n + 1) * NP, j * pb, :1],
                                in_=d_t[:, :1],
                            )

            if reps == 1:
                body()
            else:
                with tc.For_i(0, reps, 1) as i:
                    body(i)
    nc.compile()
    return nc


def _make_fast_runner(nc):
    """Cached PJRT runner: jit once, keep the inert zero output-operand
    buffers device-resident (every output element is written by the NEFF,
    so no donation/zero-init is needed), avoiding per-call re-tracing and
    the 0.5GB zero re-upload that run_bass_kernel_spmd pays."""
    import jax
    from jax.sharding import Mesh, NamedSharding, PartitionSpec
    from concourse import bass2jax

    bass2jax.install_neuronx_cc_hook()
    partition_name = nc.partition_id_tensor.name if nc.partition_id_tensor else None
    in_names, out_names, out_avals = [], [], []
    for alloc in nc.m.functions[0].allocations:
        if not isinstance(alloc, mybir.MemoryLocationSet):
            continue
        name = alloc.memorylocations[0].name
        if alloc.kind == "ExternalInput":
            if name != partition_name:
                in_names.append(name)
        elif alloc.kind == "ExternalOutput":
            out_avals.append(
                jax.core.ShapedArray(
                    tuple(alloc.tensor_shape), mybir.dt.np(alloc.dtype)
                )
            )
            out_names.append(name)
    all_in_names = list(in_names) + list(out_names)
    if partition_name is not None:
        all_in_names.append(partition_name)

    def _body(*args):
        operands = list(args)
        if partition_name is not None:
            operands.append(bass2jax.partition_id_tensor())
        return tuple(
            bass2jax._bass_exec_p.bind(
                *operands,
                out_avals=tuple(out_avals),
                in_names=tuple(all_in_names),
                out_names=tuple(out_names),
                lowering_input_output_aliases=(),
                sim_require_finite=True,
                sim_require_nnan=True,
                nc=nc,
            )
        )

    devices = jax.devices()[:N_CORES]
    mesh = Mesh(np.asarray(devices), ("core",))
    spec = PartitionSpec("core")
    fn = jax.jit(
        jax.shard_map(
            _body, mesh=mesh,
            in_specs=(spec,) * (len(in_names) + len(out_names)),
            out_specs=(spec,) * len(out_names),
            check_vma=False,
        ),
        keep_unused=True,
    )
    sharding = NamedSharding(mesh, spec)
    zeros = [
        jax.device_put(
            np.zeros((N_CORES * a.shape[0], *a.shape[1:]), a.dtype), sharding
        )
        for a in out_avals
    ]

    def run(x, act):
        by_name = {"x": x, "act": act}
        args = [jax.device_put(by_name[n], sharding) for n in in_names]
        outs = fn(*args, *zeros)
        return {n: np.asarray(o) for n, o in zip(out_names, outs)}

    return run


def kernel(x, act, batch_inds=None, _trace=False, _results_out=None, **_kw):
    import os

    x = np.ascontiguousarray(np.asarray(x, dtype=np.float32))
    act = np.ascontiguousarray(np.asarray(act, dtype=np.float32))
    assert x.shape == (B, P) and act.shape == (B, D), (x.shape, act.shape)

    if "nc" not in _cache:
        _cache["nc"] = _build()
    nc = _cache["nc"]

    g = dispatched = None
    # Fast path: cached jit runner. Skipped when a profiling harness asks
    # for traces (BASS_TRACE) so run_bass_kernel_spmd's NTFF hook can fire.
    if not _trace and not os.environ.get("BASS_TRACE"):
        try:
            if "runner" not in _cache:
                _cache["runner"] = _make_fast_runner(nc)
            out = _cache["runner"](x, act)
            g, dispatched = out["g"], out["disp"]
        except Exception:
            _cache.pop("runner", None)

    if g is None:
        in_maps = [
            {
                "x": np.ascontiguousarray(x[i * BS:(i + 1) * BS]),
                "act": np.ascontiguousarray(act[i * BS:(i + 1) * BS]),
            }
            for i in range(N_CORES)
        ]
        try:
            res = run_bass_kernel_spmd(
                nc, in_maps, list(range(N_CORES)), trace=_trace
            )
        except ModuleNotFoundError:
            # axon client without the NTFF profile hook — run untraced
            res = run_bass_kernel_spmd(
                nc, in_maps, list(range(N_CORES)), trace=False
            )
        if _results_out is not None:
            _results_out["bass_results"] = res
        g = np.concatenate([r["g"] for r in res.results], axis=0)
        dispatched = np.concatenate([r["disp"] for r in res.results], axis=0)

    mask = g >= np.float32(THRES)
    return g, mask, dispatched
